# revision 25
# baseline (speedup 1.0000x reference)
"""3-layer GAT on 8 TRN2 NeuronCores (Bass/Tile).

Strategy (dst-ownership node sharding):
  - Core c owns dst nodes [c*6250, (c+1)*6250) and processes exactly the edges
    whose dst it owns (edges sorted by dst, packed into 128-dst "windows").
  - Per layer, a gather TABLE [N+1, TBW] (bf16) holds per-node rows
    [z | el-bits(f32) | er-bits(f32)].  z/el are fetched per-edge by src via
    indirect DMA; er is fetched per-edge by dst (cheap rows).
  - Edge softmax: exp() without max-subtraction (logit ranges verified tiny;
    softmax is shift-invariant so only overflow matters).  Empty dsts handled
    via 1/max(s,eps) -> exact 0 rows like the reference.
  - Aggregation: per 128-edge block, a one-hot matrix S01[e, drel] built with
    one tensor_scalar(is_equal) vs a per-edge dstrel scalar; one PE matmul
    accumulates [dst, z*alpha | sum-alpha] into PSUM across the window.
  - Layer1 dense (z1 = x @ W1aug) is computed fully replicated on every core
    (x is replicated input; cheaper than an AllGather).  Layers 2/3 dense are
    computed on the owned shard and AllGathered into the next table.
"""

import sys
import os
import math
from contextlib import ExitStack
from dataclasses import dataclass

sys.path.insert(0, "/opt/trn_rl_repo")

import numpy as np
import ml_dtypes

BF16NP = ml_dtypes.bfloat16

import concourse.bass as bass
import concourse.bacc as bacc_mod
import concourse.mybir as mybir
import concourse.tile as tile
from concourse.bass import IndirectOffsetOnAxis, AP

F32 = mybir.dt.float32
BF = mybir.dt.bfloat16
I32 = mybir.dt.int32
AT = mybir.ActivationFunctionType
OP = mybir.AluOpType
P = 128


@dataclass
class Cfg:
    N: int = 50000
    E: int = 800000
    FIN: int = 256
    HID: int = 32
    HEADS: int = 4
    OUT: int = 40
    NEG: float = 0.2
    NC: int = 8

    @property
    def NSH(self):  # nodes per core
        return self.N // self.NC

    @property
    def NW(self):  # 128-dst windows per core
        return (self.NSH + P - 1) // P

    @property
    def LW(self):  # valid rows in last window
        return self.NSH - (self.NW - 1) * P

    @property
    def KB(self):  # k-blocks for layer-1 dense
        return self.FIN // P

    @property
    def NT1(self):  # dense tiles over all nodes (layer 1)
        return (self.N + P - 1) // P

    @property
    def NTS(self):  # dense tiles over one shard (layers 2/3)
        return self.NW

    @property
    def SLAB(self):  # windows per gather slab
        v = os.environ.get("GAT_SLAB")
        if v:
            return int(v)
        return min(4, self.NW)

    # table widths (bf16 cols)
    @property
    def TBW12(self):  # [z(128) | el 4xf32 (8) | er 4xf32 (8)]
        return P + 4 * self.HEADS

    @property
    def TBW3(self):  # [z(40) | el f32 (2) | er f32 (2)]
        return self.OUT + 4

    @property
    def GW12(self):
        return P + 2 * self.HEADS  # gathered cols: z + el bits

    @property
    def GW3(self):
        return self.OUT + 2


CFG = Cfg()


# ----------------------------------------------------------------------------
# Host-side preprocessing
# ----------------------------------------------------------------------------

def _expand_attn(a, hid, heads):
    """[heads, hid] -> [heads*hid, heads] block-diagonal map so el = z @ map."""
    m = np.zeros((heads * hid, heads), np.float32)
    for h in range(heads):
        m[h * hid:(h + 1) * hid, h] = a[h]
    return m


def _prep_host(inputs, cfg: Cfg):
    x = np.asarray(inputs["features"], np.float32)
    src = np.asarray(inputs["src"], np.int64)
    dst = np.asarray(inputs["dst"], np.int64)
    N, E, NC, NSH, NW = cfg.N, cfg.E, cfg.NC, cfg.NSH, cfg.NW

    for b in ("b1", "b2", "b3"):
        assert np.abs(np.asarray(inputs[b])).max() == 0.0, "nonzero bias unsupported"

    # ---- edge partitioning: sort by dst, split by owner, pack windows ----
    order = np.argsort(dst, kind="stable")
    s_src, s_dst = src[order], dst[order]
    core_lo = np.searchsorted(s_dst, np.arange(NC) * NSH)
    core_hi = np.searchsorted(s_dst, (np.arange(NC) + 1) * NSH)

    # window index and per-(core,window) counts
    counts = np.zeros((NC, NW), np.int64)
    for c in range(NC):
        dl = s_dst[core_lo[c]:core_hi[c]] - c * NSH
        w = dl // P
        counts[c] = np.bincount(w, minlength=NW)
    nblk = max(1, int(math.ceil(counts.max() / P)))
    wslots = nblk * P

    srcidxT, dstidxT, dstrelT = [], [], []
    for c in range(NC):
        lo, hi = core_lo[c], core_hi[c]
        dl = s_dst[lo:hi] - c * NSH
        w = dl // P
        # position of each edge within its window
        wstart = np.zeros(NW, np.int64)
        np.cumsum(counts[c][:-1], out=wstart[1:])
        pos = np.arange(hi - lo) - wstart[w]
        slot = w * wslots + pos
        si = np.full(NW * wslots, N, np.int64)
        di = np.full(NW * wslots, N, np.int64)
        dr = np.zeros(NW * wslots, np.int64)
        si[slot] = s_src[lo:hi]
        di[slot] = s_dst[lo:hi]
        dr[slot] = dl % P
        # [NW, nblk, 128] -> [128, NW*nblk]
        srcidxT.append(si.reshape(NW, nblk, P).transpose(2, 0, 1).reshape(P, -1).astype(np.int32))
        dstidxT.append(di.reshape(NW, nblk, P).transpose(2, 0, 1).reshape(P, -1).astype(np.int32))
        dstrelT.append(dr.reshape(NW, nblk, P).transpose(2, 0, 1).reshape(P, -1).astype(np.float32))

    # ---- weights ----
    hid, heads = cfg.HID, cfg.HEADS
    W1 = np.asarray(inputs["W1"], np.float32)
    W2 = np.asarray(inputs["W2"], np.float32)
    W3 = np.asarray(inputs["W3"], np.float32)
    al1 = _expand_attn(np.asarray(inputs["al1"], np.float32), hid, heads)
    ar1 = _expand_attn(np.asarray(inputs["ar1"], np.float32), hid, heads)
    al2 = _expand_attn(np.asarray(inputs["al2"], np.float32), hid, heads)
    ar2 = _expand_attn(np.asarray(inputs["ar2"], np.float32), hid, heads)
    al3 = np.asarray(inputs["al3"], np.float32).T  # [OUT, 1]
    ar3 = np.asarray(inputs["ar3"], np.float32).T

    W1aug = np.concatenate([W1, W1 @ al1, W1 @ ar1], 1)  # [FIN, 136]
    W2aug = np.concatenate([W2, W2 @ al2, W2 @ ar2], 1)  # [128, 136]
    W3aug = np.concatenate([W3, W3 @ al3, W3 @ ar3], 1)  # [128, 42]
    PW12 = W1aug.shape[1]
    w1_img = np.ascontiguousarray(
        W1aug.reshape(cfg.KB, P, PW12).transpose(1, 0, 2)).astype(BF16NP)
    w2_img = np.ascontiguousarray(
        W2aug.reshape(1, P, PW12).transpose(1, 0, 2)).astype(BF16NP)
    w3_img = np.ascontiguousarray(W3aug.reshape(P, 1, -1)).astype(BF16NP)

    # ---- replicated x, transposed+padded: xt[f, k, m] = x[m, k*128+f] ----
    npad = cfg.NT1 * P
    x_pad = np.zeros((npad, cfg.FIN), np.float32)
    x_pad[:N] = x
    xt_img = np.ascontiguousarray(
        x_pad.T.reshape(cfg.KB, P, npad).transpose(1, 0, 2)).astype(BF16NP)

    # ---- sentinel pad rows (el = -1e9 so exp -> 0) ----
    def padrow(tbw, f, nel):
        b = np.zeros(tbw * 2, np.uint8)
        b[f * 2:f * 2 + 4 * nel] = np.frombuffer(
            np.full(nel, -1e9, np.float32).tobytes(), np.uint8)
        return b.view(BF16NP).reshape(1, tbw).copy()

    pr12 = padrow(cfg.TBW12, P, heads)
    pr3 = padrow(cfg.TBW3, cfg.OUT, 1)

    iota = np.broadcast_to(np.arange(P, dtype=np.float32), (P, P)).astype(BF16NP).copy()
    ident = np.eye(P, dtype=BF16NP)

    common = dict(xt=xt_img, w1=w1_img, w2=w2_img, w3=w3_img,
                  pr12=pr12, pr3=pr3, iota=iota, ident=ident)
    in_maps = []
    for c in range(NC):
        m = dict(common)
        m.update(srcidx=srcidxT[c], dstidx=dstidxT[c], dstrel=dstrelT[c])
        in_maps.append(m)
    return in_maps, nblk


# ----------------------------------------------------------------------------
# Device program
# ----------------------------------------------------------------------------

def _bcast(ap: AP, n: int) -> AP:
    """Append a step-0 broadcast dim of size n to an AP."""
    return AP(ap.tensor, ap.offset, list(ap.ap) + [[0, n]])


def build_bass(cfg: Cfg, nblk: int) -> bass.Bass:
    N, NC, NSH, NW, LW = cfg.N, cfg.NC, cfg.NSH, cfg.NW, cfg.LW
    HEADS, OUT = cfg.HEADS, cfg.OUT
    KB, NT1 = cfg.KB, cfg.NT1
    TBW12, TBW3, GW12, GW3 = cfg.TBW12, cfg.TBW3, cfg.GW12, cfg.GW3
    PW12, PW3 = P + 2 * HEADS, OUT + 2
    BTOT = NW * nblk
    SLAB = cfg.SLAB

    nc = bacc_mod.Bacc(num_devices=NC)
    es = ExitStack()

    # ---- dram I/O ----
    xt_d = nc.declare_dram_parameter("xt", [P, KB, NT1 * P], BF, isOutput=False)
    w1_d = nc.declare_dram_parameter("w1", [P, KB, PW12], BF, isOutput=False)
    w2_d = nc.declare_dram_parameter("w2", [P, 1, PW12], BF, isOutput=False)
    w3_d = nc.declare_dram_parameter("w3", [P, 1, PW3], BF, isOutput=False)
    pr12_d = nc.declare_dram_parameter("pr12", [1, TBW12], BF, isOutput=False)
    pr3_d = nc.declare_dram_parameter("pr3", [1, TBW3], BF, isOutput=False)
    iota_d = nc.declare_dram_parameter("iota", [P, P], BF, isOutput=False)
    ident_d = nc.declare_dram_parameter("ident", [P, P], BF, isOutput=False)
    srcidx_d = nc.declare_dram_parameter("srcidx", [P, BTOT], I32, isOutput=False)
    dstidx_d = nc.declare_dram_parameter("dstidx", [P, BTOT], I32, isOutput=False)
    dstrel_d = nc.declare_dram_parameter("dstrel", [P, BTOT], F32, isOutput=False)
    out_d = nc.declare_dram_parameter("out", [NSH, OUT], F32, isOutput=True)

    tab1 = nc.dram_tensor("tab1", [N + 1, TBW12], BF)
    tab2 = nc.dram_tensor("tab2", [N + 1, TBW12], BF, addr_space="Shared")
    tab3 = nc.dram_tensor("tab3", [N + 1, TBW3], BF, addr_space="Shared")
    z2loc = nc.dram_tensor("z2loc", [NSH, TBW12], BF)
    z3loc = nc.dram_tensor("z3loc", [NSH, TBW3], BF)

    # ---- persistent sbuf ----
    srcidx_sb = es.enter_context(nc.sbuf_tensor([P, BTOT], I32))
    dstidx_sb = es.enter_context(nc.sbuf_tensor([P, BTOT], I32))
    dstrel_sb = es.enter_context(nc.sbuf_tensor([P, BTOT], F32))
    iota_sb = es.enter_context(nc.sbuf_tensor([P, P], BF))
    ident_sb = es.enter_context(nc.sbuf_tensor([P, P], BF))
    w1_sb = es.enter_context(nc.sbuf_tensor([P, KB, PW12], BF))
    w2_sb = es.enter_context(nc.sbuf_tensor([P, 1, PW12], BF))
    w3_sb = es.enter_context(nc.sbuf_tensor([P, 1, PW3], BF))
    h1T = es.enter_context(nc.sbuf_tensor([P, NW * P], BF))
    h2T = es.enter_context(nc.sbuf_tensor([P, NW * P], BF))

    # ---- preamble: constant loads + pad rows, all engines gated ----
    pre = es.enter_context(nc.semaphore("pre"))
    cnt = 0
    for dst_ap, src_ap in (
        (srcidx_sb[:], srcidx_d[:]), (dstidx_sb[:], dstidx_d[:]),
        (dstrel_sb[:], dstrel_d[:]), (iota_sb[:], iota_d[:]),
        (ident_sb[:], ident_d[:]), (w1_sb[:], w1_d[:]),
        (w2_sb[:], w2_d[:]), (w3_sb[:], w3_d[:]),
        (tab1[N:N + 1, :], pr12_d[:]), (tab2[N:N + 1, :], pr12_d[:]),
        (tab3[N:N + 1, :], pr3_d[:]),
    ):
        nc.sync.dma_start(out=dst_ap, in_=src_ap).then_inc(pre, 16)
        cnt += 16
    for eng in (nc.tensor, nc.vector, nc.scalar, nc.gpsimd, nc.sync):
        eng.wait_ge(pre, cnt)

    # ------------------------------------------------------------------
    def dense_phase(tc, layer, w_sb, kb, pw, tbw, ntiles, nrows, lhsT_of, tab_out, row0):
        """z-table tiles: psum groups of 3, copy z(bf16)+el/er(f32 bits), DMA out."""
        f = P if layer < 3 else OUT
        ew = pw - f  # f32 extra cols (el+er)
        psp, stp = pools[0]["dps"], pools[0]["dst"]
        if True:
            t = 0
            while t < ntiles:
                g = min(3, ntiles - t)
                ps = psp.tile([P, g, pw], F32, tag="dps")
                st = stp.tile([P, g, tbw], BF, tag="dst")
                for gi in range(g):
                    lhs = lhsT_of(t + gi)
                    for k in range(kb):
                        nc.tensor.matmul(out=ps[:, gi, :], lhsT=lhs[k], rhs=w_sb[:, k, :],
                                         start=(k == 0), stop=(k == kb - 1))
                # z block (bf16)
                nc.scalar.activation(out=st[:, :, 0:f], in_=ps[:, :, 0:f], func=AT.Copy)
                # el/er f32 bit-packed
                stf = st[:].bitcast(F32)  # [P, g, tbw//2]
                if layer < 3:
                    nc.scalar.activation(out=stf[:, :, f // 2:f // 2 + ew],
                                         in_=ps[:, :, f:f + ew], func=AT.Copy)
                else:
                    nc.scalar.activation(out=stf[:, :, OUT // 2:OUT // 2 + 2],
                                         in_=ps[:, :, OUT:OUT + 2], func=AT.Copy)
                # store
                r0 = row0 + t * P
                nvalid = min(g * P, nrows - t * P)
                if nvalid == g * P:
                    nc.sync.dma_start(
                        out=tab_out[r0:r0 + g * P, :].rearrange("(g p) c -> p g c", p=P),
                        in_=st[:])
                else:
                    gfull = nvalid // P
                    if gfull:
                        nc.sync.dma_start(
                            out=tab_out[r0:r0 + gfull * P, :].rearrange("(g p) c -> p g c", p=P),
                            in_=st[:, 0:gfull, :])
                    rem = nvalid - gfull * P
                    if rem:
                        nc.sync.dma_start(
                            out=tab_out[r0 + gfull * P:r0 + nvalid, :],
                            in_=st[0:rem, gfull, :])
                t += g

    # ------------------------------------------------------------------
    def edge_phase(tc, layer, tab, tbw, gw, f, h, eoff, erw, hT_out):
        neg = cfg.NEG
        fh = f // h
        gzp, gerp, msgp, smp = pools[0]["gz"], pools[0]["ger"], pools[0]["msg"], pools[0]["sm"]
        s01p, hvp, psp, ptp = pools[0]["s01"], pools[0]["hv"], pools[0]["eps"], pools[0]["ept"]
        if True:
            for s0 in range(0, NW, SLAB):
                sl = min(SLAB, NW - s0)
                gz = gzp.tile([P, sl * nblk, gw], BF, tag="gz")
                ger = gerp.tile([P, sl * nblk, erw], BF, tag="ger")
                c0, c1 = s0 * nblk, (s0 + sl) * nblk
                for ci in range(c0, c1):
                    nc.gpsimd.indirect_dma_start(
                        out=gz[:, ci - c0, :], out_offset=None, in_=tab[:],
                        in_offset=IndirectOffsetOnAxis(ap=srcidx_sb[:, ci:ci + 1], axis=0))
                    nc.gpsimd.indirect_dma_start(
                        out=ger[:, ci - c0, :], out_offset=None, in_=tab[:],
                        in_offset=IndirectOffsetOnAxis(ap=dstidx_sb[:, ci:ci + 1], axis=0),
                        element_offset=eoff)
                if s0 == 0 and layer == 1 and os.environ.get("GAT_DEBUG") == "gz":
                    nc.sync.dma_start(out=out_d[0:P, :].bitcast(BF),
                                      in_=gz[:, 0, 0:2 * OUT])
                    nc.sync.dma_start(out=out_d[P:P + P, 0:4].bitcast(BF),
                                      in_=gz[:, 0, P:P + 2 * HEADS])
                for wr in range(sl):
                    w = s0 + wr
                    b0 = wr * nblk
                    el = gz[:].bitcast(F32)[:, b0:b0 + nblk, gw // 2 - h:gw // 2]
                    er = ger[:].bitcast(F32)[:, b0:b0 + nblk, 0:h]
                    ef = smp.tile([P, nblk, h], F32, tag="ef")
                    nc.vector.tensor_tensor(out=ef[:], in0=el, in1=er, op=OP.add)
                    nc.vector.scalar_tensor_tensor(out=ef[:], in0=ef[:], scalar=neg,
                                                   in1=ef[:], op0=OP.mult, op1=OP.max)
                    ee = smp.tile([P, nblk, h], BF, tag="ee")
                    nc.scalar.activation(out=ee[:], in_=ef[:], func=AT.Exp)
                    msgt = msgp.tile([P, nblk, f + h], BF, tag="msgt")
                    nc.vector.tensor_tensor(
                        out=msgt[:, :, 0:f].rearrange("p b (h f) -> p b h f", h=h),
                        in0=gz[:, b0:b0 + nblk, 0:f].rearrange("p b (h f) -> p b h f", h=h),
                        in1=_bcast(ee[:], fh), op=OP.mult)
                    nc.vector.tensor_copy(out=msgt[:, :, f:f + h], in_=ee[:])
                    ps = psp.tile([P, f + h], F32, tag="eps")
                    for b in range(nblk):
                        s01 = s01p.tile([P, P], BF, tag="s01")
                        nc.vector.tensor_scalar(
                            out=s01[:], in0=iota_sb[:],
                            scalar1=dstrel_sb[:, w * nblk + b:w * nblk + b + 1],
                            scalar2=None, op0=OP.is_equal)
                        nc.tensor.matmul(out=ps[:], lhsT=s01[:], rhs=msgt[:, b, :],
                                         start=(b == 0), stop=(b == nblk - 1))
                    # ---- epilogue ----
                    sc = smp.tile([P, h], F32, tag="sc")
                    nc.vector.tensor_scalar(out=sc[:], in0=ps[:, f:f + h],
                                            scalar1=1e-30, scalar2=None, op0=OP.max)
                    rcp = smp.tile([P, h], F32, tag="rcp")
                    nc.vector.reciprocal(out=rcp[:], in_=sc[:])
                    if layer < 3:
                        hv = hvp.tile([P, f], BF, tag="hv")
                        nc.vector.tensor_tensor(
                            out=hv[:].rearrange("p (h f) -> p h f", h=h),
                            in0=ps[:, 0:f].rearrange("p (h f) -> p h f", h=h),
                            in1=_bcast(rcp[:], fh), op=OP.mult)
                        nc.vector.tensor_scalar(out=hv[:], in0=hv[:], scalar1=0.0,
                                                scalar2=None, op0=OP.max)
                        pt = ptp.tile([P, P], BF, tag="pt")
                        nc.tensor.transpose(out=pt[:], in_=hv[:], identity=ident_sb[:])
                        nc.scalar.activation(out=hT_out[:, w * P:(w + 1) * P],
                                             in_=pt[:], func=AT.Copy)
                    else:
                        o = hvp.tile([P, OUT], F32, tag="o40")
                        nc.vector.tensor_scalar(out=o[:], in0=ps[:, 0:OUT],
                                                scalar1=rcp[:, 0:1], scalar2=None,
                                                op0=OP.mult)
                        rmax = smp.tile([P, 1], F32, tag="rmax")
                        nc.vector.tensor_reduce(out=rmax[:], in_=o[:],
                                                axis=mybir.AxisListType.X, op=OP.max)
                        xm = hvp.tile([P, OUT], F32, tag="xm")
                        nc.vector.tensor_scalar(out=xm[:], in0=o[:], scalar1=rmax[:, 0:1],
                                                scalar2=None, op0=OP.subtract)
                        pex = hvp.tile([P, OUT], F32, tag="pex")
                        ssum = smp.tile([P, 1], F32, tag="ssum")
                        nc.scalar.activation(out=pex[:], in_=xm[:], func=AT.Exp,
                                             accum_out=ssum[:])
                        lg = smp.tile([P, 1], F32, tag="lg")
                        nc.scalar.activation(out=lg[:], in_=ssum[:], func=AT.Ln)
                        res = hvp.tile([P, OUT], F32, tag="res")
                        nc.vector.tensor_scalar(out=res[:], in0=xm[:], scalar1=lg[:, 0:1],
                                                scalar2=None, op0=OP.subtract)
                        rows = P if w < NW - 1 else LW
                        nc.sync.dma_start(out=out_d[w * P:w * P + rows, :],
                                          in_=res[0:rows, :])

    # ------------------------------------------------------------------
    # One TileContext; phases separated by strict all-engine barriers so the
    # untracked DRAM-table and raw-sbuf (h1T/h2T) producer->consumer edges
    # are ordered, and collectives sit between barriers.
    pools = [None]

    def mk_pools(tc, ps_):
        pools[0] = dict(
            dps=ps_.enter_context(tc.tile_pool(name="dps", bufs=2, space="PSUM")),
            dst=ps_.enter_context(tc.tile_pool(name="dst", bufs=2)),
            gz=ps_.enter_context(tc.tile_pool(name="gz", bufs=2)),
            ger=ps_.enter_context(tc.tile_pool(name="ger", bufs=2)),
            msg=ps_.enter_context(tc.tile_pool(name="msg", bufs=2)),
            sm=ps_.enter_context(tc.tile_pool(name="sm", bufs=3)),
            s01=ps_.enter_context(tc.tile_pool(name="s01", bufs=3)),
            hv=ps_.enter_context(tc.tile_pool(name="hv", bufs=2)),
            eps=ps_.enter_context(tc.tile_pool(name="eps", bufs=2, space="PSUM")),
            ept=ps_.enter_context(tc.tile_pool(name="ept", bufs=2, space="PSUM")),
        )
        return pools[0]

    with tile.TileContext(nc) as tc, ExitStack() as ps_:
        mk_pools(tc, ps_)
        # ---- layer-1 dense, replicated over all nodes ----
        with tc.tile_pool(name="xload", bufs=2) as xlp:
            SLT = 15  # tiles per x slab (multiple of the psum group size 3)
            state = {}

            def lhsT_of(t):
                sl = t // SLT
                if state.get("sl") != sl:
                    g = min(SLT, NT1 - sl * SLT)
                    xb = xlp.tile([P, KB, SLT * P], BF, tag="xb")
                    nc.sync.dma_start(out=xb[:, :, 0:g * P],
                                      in_=xt_d[:, :, sl * SLT * P:(sl * SLT + g) * P])
                    state.update(sl=sl, xb=xb)
                xb = state["xb"]
                ti = t % SLT
                return [xb[:, k, ti * P:(ti + 1) * P] for k in range(KB)]

            dense_phase(tc, 1, w1_sb, KB, PW12, TBW12, NT1, N, lhsT_of, tab1, 0)

        tc.strict_bb_all_engine_barrier()
        edge_phase(tc, 1, tab1, TBW12, GW12, P, HEADS, P + 2 * HEADS, 2 * HEADS, h1T)
        tc.strict_bb_all_engine_barrier()
        dense_phase(tc, 2, w2_sb, 1, PW12, TBW12, cfg.NTS, NSH,
                    lambda t: [h1T[:, t * P:(t + 1) * P]], z2loc, 0)

    if os.environ.get("GAT_DEBUG") in ("h1", "gz"):
        if os.environ.get("GAT_DEBUG") == "h1":
            dbg = es.enter_context(nc.semaphore("dbg"))
            with nc.allow_non_contiguous_dma(reason="debug tap"):
                nc.sync.dma_start(out=out_d[:].bitcast(BF).rearrange("n f -> f n"),
                                  in_=h1T[0:2 * OUT, 0:NSH]).then_inc(dbg, 16)
            nc.sync.wait_ge(dbg, 16)
        es.close()
        nc.compile()
        return nc

    if os.environ.get("GAT_DEBUG") == "tab1":
        dbg = es.enter_context(nc.semaphore("dbg"))
        nc.sync.dma_start(out=out_d[:].bitcast(BF),
                          in_=tab1[0:NSH, 0:2 * OUT]).then_inc(dbg, 16)
        nc.sync.wait_ge(dbg, 16)
        es.close()
        nc.compile()
        return nc

    # ---- AllGather 1 (raw, between TileContexts; explicit completion sem) ----
    cc1 = es.enter_context(nc.semaphore("cc1"))
    nc.gpsimd.collective_compute(
        "AllGather", OP.bypass, replica_groups=[list(range(NC))],
        ins=[z2loc[:]], outs=[tab2[0:N, :]]).then_inc(cc1, 1)
    for eng in (nc.tensor, nc.vector, nc.scalar, nc.gpsimd, nc.sync):
        eng.wait_ge(cc1, 1)

    with tile.TileContext(nc) as tc, ExitStack() as ps_:
        mk_pools(tc, ps_)
        edge_phase(tc, 2, tab2, TBW12, GW12, P, HEADS, P + 2 * HEADS, 2 * HEADS, h2T)
        tc.strict_bb_all_engine_barrier()
        dense_phase(tc, 3, w3_sb, 1, PW3, TBW3, cfg.NTS, NSH,
                    lambda t: [h2T[:, t * P:(t + 1) * P]], z3loc, 0)

    cc2 = es.enter_context(nc.semaphore("cc2"))
    nc.gpsimd.collective_compute(
        "AllGather", OP.bypass, replica_groups=[list(range(NC))],
        ins=[z3loc[:]], outs=[tab3[0:N, :]]).then_inc(cc2, 1)
    for eng in (nc.tensor, nc.vector, nc.scalar, nc.gpsimd, nc.sync):
        eng.wait_ge(cc2, 1)

    with tile.TileContext(nc) as tc, ExitStack() as ps_:
        mk_pools(tc, ps_)
        edge_phase(tc, 3, tab3, TBW3, GW3, OUT, 1, OUT + 2, 2, None)

    es.close()
    nc.compile()
    return nc


# ----------------------------------------------------------------------------

_CACHE = {}


def _run(inputs, cfg: Cfg):
    in_maps, nblk = _prep_host(inputs, cfg)
    key = (cfg.N, cfg.E, nblk)
    if key not in _CACHE:
        _CACHE[key] = build_bass(cfg, nblk)
    nc = _CACHE[key]
    from concourse.bass_utils import run_bass_kernel_spmd
    res = run_bass_kernel_spmd(nc, in_maps, list(range(cfg.NC)))
    outs = [np.asarray(res.results[c]["out"], np.float32) for c in range(cfg.NC)]
    return np.concatenate(outs, 0), res


def kernel(**inputs) -> np.ndarray:
    out, _ = _run(inputs, CFG)
    return out
